# revision 1
# baseline (speedup 1.0000x reference)
"""Trainium2 Bass kernel: 3D trilinear grid_sample with strict-bounds masking
(nn_DenseMotionNetwork). Data-parallel over batch N=8 -> 8 NeuronCores.

v2 design:
- 512B gather rows: table of 65536 cells (256B each); cell c=(z*64+y)*64+x
  holds elem[ch*4 + zz*2 + yy] = vol[ch, z+zz, y+yy, x] (zero-padded edges).
  A point's row = cells (zc,yc,xc) and (zc,yc,xc+1), read as one 512B
  transfer via elem_step=128 (overlapping rows, stride 256B).
- int16 index wrap: cell ids >= 32768 are stored as negative i16; the gather
  ucode sign-extends and multiplies signed (IVP_MULUSAN), so the table is
  laid out split: buffer[0:8MiB] = cells 32768.., buffer[8MiB:16MiB] =
  cells 0..32767, and the gather base AP points at buffer + 8MiB.
- trailing-trim guard: the ucode drops trailing negative indices, so each
  gather's index list is CHUNK real points + 1 dummy index (cell 0).
- 4 SWDGE queues round-robin (chunk k -> queue k%4): descriptor generation
  runs on all 8 Q7 cores (2 per queue) concurrently. Queue q's cores read
  indices from partition groups 2q and 2q+1, so the host lays out a second
  copy of the grid in exactly that wrapped layout and the cell-index
  pipeline computes indices in-place -- no partition-shuffle DMAs.
- combine: P8 = G * coef8 (one DVE mult, coef broadcast over channels),
  then one tensor_reduce over (xx, zy) -> [128, CB, 32] f32, DMA'd out
  point-major; the host does the final transpose to (C, D, H, W).
"""
import numpy as np

C = 32
ID, IH, IW = 16, 64, 64
N_CORES = 8
P = ID * IH * IW                     # 65536 points per batch element
NCELL = ID * IH * IW                 # 65536 cells
CELL_ELEMS = 128                     # 256B per cell (32ch x 2z x 2y), bf16
ROW_ELEMS = 256                      # 512B per gathered row (2 cells)
CHUNK = 896                          # points per gather (+1 dummy idx)
CBM = CHUNK // 128                   # 7 column groups per main chunk
NMAIN = 73                           # 73*896 = 65408
TAIL = P - NMAIN * CHUNK             # 128
NCHUNK = NMAIN + 1                   # 74
NCOL = P // 128                      # 512 grid columns (coef layout)
WCH = 57                             # idx cols per chunk (897 = 56*16+1)
NBATCH = (NCHUNK + 3) // 4           # 19 chunk batches per queue
NBW = WCH * NBATCH                   # 1083 idx-layout columns
OUTW = NMAIN * CBM * C + (TAIL // 128) * C   # 16384 output cols
NQ = 4                               # SWDGE queues used
# grid coords that map to cell 0 with all-masked weights (dummy slots)
DUMX = -31.5 / 32.0
DUMZ = -7.5 / 8.0

_COMPILED = {}


def _build():
    import concourse.bass as bass
    import concourse.bacc as bacc
    import concourse.mybir as mybir
    from concourse.ap import AP
    from concourse.mybir import AluOpType as ALU
    from concourse.mybir import ActivationFunctionType as ACTF
    from concourse.tile import TileContext

    FP32 = mybir.dt.float32
    BF16 = mybir.dt.bfloat16
    I16 = mybir.dt.int16
    I32 = mybir.dt.int32

    nc = bacc.Bacc("TRN2", debug=False, num_swdge_queues=NQ,
                   dynamic_dma_scratch_size=49152)
    table = nc.dram_tensor("table", [NCELL + 1, CELL_ELEMS], BF16,
                           kind="ExternalInput")
    gxs = nc.dram_tensor("gxs", [128, NCOL], FP32, kind="ExternalInput")
    gys = nc.dram_tensor("gys", [128, NCOL], FP32, kind="ExternalInput")
    gzs = nc.dram_tensor("gzs", [128, NCOL], FP32, kind="ExternalInput")
    gbx = nc.dram_tensor("gbx", [128, NBW], FP32, kind="ExternalInput")
    gby = nc.dram_tensor("gby", [128, NBW], FP32, kind="ExternalInput")
    gbz = nc.dram_tensor("gbz", [128, NBW], FP32, kind="ExternalInput")
    outp = nc.dram_tensor("outp", [128, OUTW], BF16, kind="ExternalOutput")

    # gather source: base at cell 32768 (buffer middle); signed i16 idx *
    # 256B stride reaches [-8MiB, +8MiB) around it = the whole table.
    gather_src = AP(table, 32768 * CELL_ELEMS, [[CELL_ELEMS, 32768],
                                                [1, ROW_ELEMS]])

    with TileContext(nc) as tc:
        with (
            tc.tile_pool(name="persist", bufs=1) as persist,
            tc.tile_pool(name="pw", bufs=1) as pw,
            tc.tile_pool(name="gather", bufs=8) as gpool,
            tc.tile_pool(name="combine", bufs=3) as cpool,
            tc.tile_pool(name="outs", bufs=4) as opool,
        ):
            gbx_sb = persist.tile([128, NBW], FP32, name="gbx_sb")
            gby_sb = persist.tile([128, NBW], FP32, name="gby_sb")
            gbz_sb = persist.tile([128, NBW], FP32, name="gbz_sb")
            nc.sync.dma_start(gbx_sb[:, :], gbx.ap())
            nc.sync.dma_start(gby_sb[:, :], gby.ap())
            nc.sync.dma_start(gbz_sb[:, :], gbz.ap())
            gx_sb = persist.tile([128, NCOL], FP32, name="gx_sb")
            gy_sb = persist.tile([128, NCOL], FP32, name="gy_sb")
            gz_sb = persist.tile([128, NCOL], FP32, name="gz_sb")
            nc.sync.dma_start(gx_sb[:, :], gxs.ap())
            nc.sync.dma_start(gy_sb[:, :], gys.ap())
            nc.sync.dma_start(gz_sb[:, :], gzs.ap())

            coef8 = persist.tile([128, NCOL, 8], BF16, name="coef8")
            idx16 = persist.tile([128, NBW], I16, name="idx16")

            # ---- index pipeline (wrapped idx layout, [128, NBW]) ----
            def tb(nm):
                return pw.tile([128, NBW], FP32, name=nm, tag=nm)

            icvtB = pw.tile([128, NBW], I32, name="icvtB", tag="icvtB")
            rfB, gttB, vB = tb("rfB"), tb("gttB"), tb("vB")
            cellB, vcB = tb("cellB"), tb("vcB")

            def floor_clamped(dst, src_sb, scale, bias, hi):
                # dst = clamp(floor(src*scale + bias), 0, hi)
                nc.scalar.activation(vB[:, :], src_sb[:, :], ACTF.Copy,
                                     bias=bias, scale=scale)
                nc.vector.tensor_copy(icvtB[:, :], vB[:, :])
                nc.vector.tensor_copy(rfB[:, :], icvtB[:, :])
                nc.vector.tensor_tensor(gttB[:, :], rfB[:, :], vB[:, :],
                                        ALU.is_gt)
                nc.vector.tensor_tensor(dst[:, :], rfB[:, :], gttB[:, :],
                                        ALU.subtract)
                nc.vector.tensor_scalar(dst[:, :], dst[:, :], 0.0, hi,
                                        ALU.max, ALU.min)

            floor_clamped(cellB, gbz_sb, 8.0, 7.5, 15.0)       # zc
            floor_clamped(vcB, gby_sb, 32.0, 31.5, 63.0)       # yc
            nc.vector.scalar_tensor_tensor(cellB[:, :], cellB[:, :], 64.0,
                                           vcB[:, :], ALU.mult, ALU.add)
            floor_clamped(vcB, gbx_sb, 32.0, 31.5, 63.0)       # xc
            nc.vector.scalar_tensor_tensor(cellB[:, :], cellB[:, :], 64.0,
                                           vcB[:, :], ALU.mult, ALU.add)
            nc.vector.tensor_scalar(rfB[:, :], cellB[:, :], 32768.0, None,
                                    ALU.is_ge)
            nc.vector.scalar_tensor_tensor(cellB[:, :], rfB[:, :], -65536.0,
                                           cellB[:, :], ALU.mult, ALU.add)
            nc.vector.tensor_copy(idx16[:, :], cellB[:, :])

            # ---- coefficient pipeline (natural layout, [128, NCOL]) ----
            def t(nm):
                return pw.tile([128, NCOL], FP32, name=nm, tag=nm)

            ix, iy, iz = t("ix"), t("iy"), t("iz")
            nc.scalar.activation(ix[:, :], gx_sb[:, :], ACTF.Copy,
                                 bias=31.5, scale=32.0)
            nc.scalar.activation(iy[:, :], gy_sb[:, :], ACTF.Copy,
                                 bias=31.5, scale=32.0)
            nc.scalar.activation(iz[:, :], gz_sb[:, :], ACTF.Copy,
                                 bias=7.5, scale=8.0)

            icvt = pw.tile([128, NCOL], I32, name="icvt", tag="icvt")
            rf, gtt = t("rf"), t("gtt")
            fx, fy, fz = t("fx"), t("fy"), t("fz")
            x0, y0, z0 = t("x0"), t("y0"), t("z0")

            def floor_to(dst, frac, src):
                nc.vector.tensor_copy(icvt[:, :], src[:, :])
                nc.vector.tensor_copy(rf[:, :], icvt[:, :])
                nc.vector.tensor_tensor(gtt[:, :], rf[:, :], src[:, :],
                                        ALU.is_gt)
                nc.vector.tensor_tensor(dst[:, :], rf[:, :], gtt[:, :],
                                        ALU.subtract)
                nc.vector.tensor_tensor(frac[:, :], src[:, :], dst[:, :],
                                        ALU.subtract)

            floor_to(x0, fx, ix)
            floor_to(y0, fy, iy)
            floor_to(z0, fz, iz)

            # strict-bounds masked 1D weights:
            # w0 = (1-f)*[(v>0)&(v<HI)], w1 = f*[(v+1>0)&(v+1<HI)]
            wpair = {}
            ma, mb = t("ma"), t("mb")
            for nm, v0, f, hiv in (("x", x0, fx, 64.0), ("y", y0, fy, 64.0),
                                   ("z", z0, fz, 16.0)):
                m0, m1 = t(nm + "m0"), t(nm + "m1")
                nc.vector.tensor_scalar(ma[:, :], v0[:, :], 0.0, None,
                                        ALU.is_gt)
                nc.vector.tensor_scalar(mb[:, :], v0[:, :], hiv, None,
                                        ALU.is_lt)
                nc.vector.tensor_tensor(m0[:, :], ma[:, :], mb[:, :], ALU.mult)
                nc.vector.tensor_scalar(ma[:, :], v0[:, :], -0.5, None,
                                        ALU.is_gt)
                nc.vector.tensor_scalar(mb[:, :], v0[:, :], hiv - 1.0, None,
                                        ALU.is_lt)
                nc.vector.tensor_tensor(m1[:, :], ma[:, :], mb[:, :], ALU.mult)
                w0, w1, fneg = t(nm + "w0"), t(nm + "w1"), t("fn")
                nc.vector.tensor_scalar(fneg[:, :], f[:, :], -1.0, 1.0,
                                        ALU.mult, ALU.add)
                nc.vector.tensor_tensor(w0[:, :], fneg[:, :], m0[:, :],
                                        ALU.mult)
                nc.vector.tensor_tensor(w1[:, :], f[:, :], m1[:, :], ALU.mult)
                wpair[nm] = (w0, w1)

            # coef8[xx*4 + zz*2 + yy] = wx[xx]*wz[zz]*wy[yy] (bf16 out)
            wx, wy, wz = wpair["x"], wpair["y"], wpair["z"]
            for zz in (0, 1):
                for yy in (0, 1):
                    czy = t(f"czy{zz}{yy}")
                    nc.vector.tensor_tensor(czy[:, :], wz[zz][:, :],
                                            wy[yy][:, :], ALU.mult)
                    for xx in (0, 1):
                        nc.vector.tensor_tensor(
                            coef8[:, :, xx * 4 + zz * 2 + yy],
                            czy[:, :], wx[xx][:, :], ALU.mult)

            # ---- gather + combine loop ----
            for k in range(NCHUNK):
                tail = (k == NMAIN)
                cb = (TAIL // 128) if tail else CBM
                nidx = cb * 128 + 1
                tcol = (k // NQ) * WCH
                G = gpool.tile([128, 8, ROW_ELEMS], BF16, name="G", tag="G")
                ogroups = 2 if tail else 8
                nc.gpsimd.dma_gather(
                    G[:, 0:ogroups, :],
                    gather_src,
                    idx16[:, tcol:tcol + (nidx + 15) // 16],
                    nidx, nidx, ROW_ELEMS,
                    elem_step=CELL_ELEMS,
                    queue_num=k % NQ,
                )

                # ISA free-dim limit is 3: merge (b, xx) into one dim
                P8 = cpool.tile([128, 2 * CBM, C, 4], BF16, name="P8",
                                tag="P8")
                gv = G[:, 0:cb, :].rearrange(
                    "p b (xx ch zy) -> p (b xx) ch zy", xx=2, ch=C)
                cv = (coef8[:, k * CBM:k * CBM + cb, :]
                      .rearrange("p w (xx zy) -> p (w xx) zy", xx=2)
                      .unsqueeze(2)
                      .broadcast_to((128, 2 * cb, C, 4)))
                nc.vector.tensor_tensor(P8[:, 0:2 * cb, :, :], gv, cv,
                                        ALU.mult)
                # contiguous bf16 fold tree (2x-mode eligible) instead of
                # 1x-mode tensor_reduce: xx fold, then zy pair folds
                pv = P8.rearrange("p (b xx) ch zy -> p b xx ch zy", xx=2)
                X1 = cpool.tile([128, CBM, C, 4], BF16, name="X1", tag="X1")
                nc.vector.tensor_tensor(X1[:, 0:cb, :, :],
                                        pv[:, 0:cb, 0, :, :],
                                        pv[:, 0:cb, 1, :, :], ALU.add)
                Z1 = cpool.tile([128, CBM, C, 2], BF16, name="Z1", tag="Z1")
                nc.vector.tensor_tensor(Z1[:, 0:cb, :, :],
                                        X1[:, 0:cb, :, 0:2],
                                        X1[:, 0:cb, :, 2:4], ALU.add)
                F = opool.tile([128, CBM, C], BF16, name="F", tag="F")
                nc.vector.tensor_tensor(F[:, 0:cb, :], Z1[:, 0:cb, :, 0],
                                        Z1[:, 0:cb, :, 1], ALU.add)
                col0 = k * CBM * C
                nc.sync.dma_start(outp.ap()[:, col0:col0 + cb * C],
                                  F[:, 0:cb, :])
    nc.compile()
    return nc


def _prep_core_inputs(inp_n, grid_n):
    """inp_n (C, ID, IH, IW) f32, grid_n (D, H, W, 3) f32 -> input dict."""
    import ml_dtypes
    from numpy.lib.stride_tricks import as_strided
    bf16 = ml_dtypes.bfloat16
    volp = np.zeros((C, ID + 1, IH + 1, IW), dtype=bf16)
    volp[:, :ID, :IH, :] = inp_n.astype(bf16)
    sC, sZ, sY, sX = volp.strides
    cells = as_strided(volp, shape=(ID, IH, IW, C, 2, 2),
                       strides=(sZ, sY, sX, sC, sZ, sY))
    cells = np.ascontiguousarray(cells).reshape(NCELL, CELL_ELEMS)
    buf = np.empty((NCELL + 1, CELL_ELEMS), dtype=bf16)
    half = NCELL // 2
    buf[0:half] = cells[half:NCELL]
    buf[half:NCELL] = cells[0:half]
    buf[NCELL] = 0

    g = grid_n.reshape(-1, 3).astype(np.float32)
    # idx-layout grids: chunk k (queue q=k%4, batch t=k//4) occupies
    # partition groups 2q..2q+1 (rows 32q..32q+31, both 16-row halves get
    # the same block) at cols [57t, 57t+57); block[w, plo] = point
    # k*896 + w*16 + plo, dummy-padded.
    gb = np.empty((3, 128, NBW), dtype=np.float32)
    gb[0] = DUMX
    gb[1] = DUMX
    gb[2] = DUMZ
    pts = np.full((NCHUNK, WCH * 16), 0, dtype=np.int64)
    for k in range(NCHUNK):
        npts = TAIL if k == NMAIN else CHUNK
        pts[k, :npts] = k * CHUNK + np.arange(npts)
        pts[k, npts:] = -1
    for k in range(NCHUNK):
        q, tt = k % NQ, k // NQ
        blk = pts[k].reshape(WCH, 16)        # [w, plo]
        val = np.where(blk[None] >= 0,
                       g[blk.clip(0), [[[0]], [[1]], [[2]]]],
                       np.array([DUMX, DUMX, DUMZ])[:, None, None])
        for half16 in (0, 1):
            r0 = 32 * q + 16 * half16
            gb[:, r0:r0 + 16, WCH * tt:WCH * (tt + 1)] = (
                val.transpose(0, 2, 1))
    return {
        "table": buf,
        "gxs": np.ascontiguousarray(g[:, 0].reshape(NCOL, 128).T),
        "gys": np.ascontiguousarray(g[:, 1].reshape(NCOL, 128).T),
        "gzs": np.ascontiguousarray(g[:, 2].reshape(NCOL, 128).T),
        "gbx": np.ascontiguousarray(gb[0]),
        "gby": np.ascontiguousarray(gb[1]),
        "gbz": np.ascontiguousarray(gb[2]),
    }


def _decode_out(o):
    """outp (128, OUTW) bf16 -> (C, ID, IH, IW); point p = 896k + 128b + r."""
    o = np.asarray(o, dtype=np.float32)
    main = o[:, :NMAIN * CBM * C].reshape(128, NMAIN, CBM, C)
    main = main.transpose(3, 1, 2, 0).reshape(C, NMAIN * CHUNK)
    tail = o[:, NMAIN * CBM * C:].reshape(128, C).T
    return np.concatenate([main, tail], axis=1).reshape(C, ID, IH, IW)


def _get_compiled(key="default"):
    if key not in _COMPILED:
        _COMPILED[key] = _build()
    return _COMPILED[key]


def _run(inputs, trace=False, core_ids=None):
    """Returns (output (8,32,16,64,64) f32, BassKernelResults)."""
    from concourse import bass_utils

    inp = np.asarray(inputs["input"], dtype=np.float32)
    grid = np.asarray(inputs["grid"], dtype=np.float32)
    ac = int(np.asarray(inputs["align_corners"]))
    assert ac == 0, "kernel specialized for align_corners=0"
    N = inp.shape[0]
    if core_ids is None:
        core_ids = list(range(N))

    nc = _get_compiled()
    in_maps = [_prep_core_inputs(inp[n], grid[n]) for n in range(len(core_ids))]
    res = bass_utils.run_bass_kernel_spmd(nc, in_maps, core_ids=core_ids,
                                          trace=trace)
    out = np.empty((len(core_ids), C, ID, IH, IW), dtype=np.float32)
    for n in range(len(core_ids)):
        out[n] = _decode_out(res.results[n]["outp"])
    return out, res


def kernel(**inputs):
    out, _ = _run(inputs, trace=False)
    return out

